# revision 14
# baseline (speedup 1.0000x reference)
"""LogHarmonicLowering Trainium2 kernel.

Computes out[b, k*C + c, f, t] = W1[k,f] * x[b, c, i0[k,f], t] + W2[k,f] * x[b, c, i0[k,f]+1, t]
for K=5 log-harmonic frequency shifts, where i0/W1/W2 replicate the reference's
float32 fractional-shift linear interpolation (with zero padding past the top
of the frequency axis).

Because each shift k is a constant scalar, i0[k,f] = f + m_k (+1 at rare f32
rounding anomalies, folded into the weight tables), so each output block is an
integer row shift plus a constant-per-f blend of two adjacent frequency rows.
k = 0 is an exact copy, done host-side; the device computes k = 1..4.

Distribution: pure data parallel. The 64 (b, c) slices are split 8 per core.
Per core/slice, frequency lives on SBUF partitions as blocks of 128 rows with
T=512 on the free dim. Output tiles for shift k are realigned by r_k = m_k % 128
so the main operand is tile-aligned; the +1-row-shifted operand X1 is built on
the otherwise-idle TensorEngine as a bit-exact fp32 permutation matmul into
PSUM (compute engines cannot do partition-misaligned SBUF access, and re-reading
the shifted copy from HBM would add 14 MiB/core to a DMA-bound kernel).
All HBM arrays are laid out [.., partition, block, t] so every DMA is one
contiguous ~14 KB descriptor per partition (descriptor-generation bound
otherwise); the host pre/post transposes, which is cheap next to device time.
Per (slice, k, block): ScalarE computes bt = B ⊙ X1 (per-partition weights),
VectorE computes out = (X ⊙ A) + bt in one fused scalar_tensor_tensor op.
"""

import numpy as np
from functools import lru_cache

P = 128          # SBUF partitions
FR = 1024        # frequency bins
T = 512          # time steps
NB = FR // P     # 8 frequency blocks per slice
NBL = NB - 1     # blocks 1..7 are the only ones device kernels read
NS = 8           # (b, c) slices per core
NCORES = 8
B_DIM, C_DIM, K = 4, 16, 5

F_KERNEL_SIZE = 5
ANCHOR = 1
OUT_LOG_SCALE = 200.0
IN_LOG_SCALE = 0.001


def _shifts_f32() -> np.ndarray:
    """Reference's make_log_shift(), cast to float32 exactly as jnp.asarray does."""
    np_shift = (np.arange(F_KERNEL_SIZE) + 1) / ANCHOR
    log_shift = OUT_LOG_SCALE * np.log(IN_LOG_SCALE * np_shift)
    log_shift -= log_shift[ANCHOR - 1]
    return (-log_shift).astype(np.float32)


def _weight_tables():
    """Per-f effective weights A, B (float32) such that
    out[f] = A[k,f] * x[f + m_k] + B[k,f] * x[f + m_k + 1],
    bit-replicating the reference's f32 pos/floor/mask arithmetic."""
    shifts = _shifts_f32()                                     # (K,)
    f = np.arange(FR, dtype=np.float32)
    pos = f[None, :] - shifts[:, None]                         # f32 (K, FR)
    i0 = np.floor(pos).astype(np.int32)
    w = (pos - i0.astype(np.float32)).astype(np.float32)
    v0 = ((i0 >= 0) & (i0 < FR)).astype(np.float32)
    v1 = ((i0 + 1 >= 0) & (i0 + 1 < FR)).astype(np.float32)
    w1 = ((np.float32(1.0) - w) * v0).astype(np.float32)       # weight of x[i0]
    w2 = (w * v1).astype(np.float32)                           # weight of x[i0+1]

    m = i0[:, 0].copy()                                        # integer base shift per k
    delta = i0 - (np.arange(FR, dtype=np.int64)[None, :] + m[:, None])
    assert np.all((delta == 0) | (delta == 1))
    # where the f32 sum rounded up to the next integer, w == 0 exactly:
    assert np.all(w2[delta == 1] == 0.0)
    A = np.where(delta == 0, w1, np.float32(0.0)).astype(np.float32)
    B = np.where(delta == 0, w2, w1).astype(np.float32)
    assert m[0] == 0 and np.all(A[0] == 1.0) and np.all(B[0] == 0.0)
    return m, A, B


_M, _A, _B = _weight_tables()
Q = [int(_M[k]) // P for k in range(K)]       # [0, 1, 1, 2, 2]
R = [int(_M[k]) % P for k in range(K)]        # [0, 10, 91, 21, 65]
# number of output blocks computed per k (k >= 1): source block fo+q must be < NB
NBLK = [NB - Q[k] for k in range(K)]


def _packed_weights():
    """Pack A/B into [P, ncols] tiles: column (k, fo) holds the weight for
    output partition p at f = 128*fo - r_k + p (zero outside [0, FR))."""
    cols = [(k, fo) for k in range(1, K) for fo in range(NBLK[k])]
    WA = np.zeros((P, len(cols)), dtype=np.float32)
    WB = np.zeros((P, len(cols)), dtype=np.float32)
    for ci, (k, fo) in enumerate(cols):
        fvals = P * fo - R[k] + np.arange(P)
        ok = (fvals >= 0) & (fvals < FR)
        WA[ok, ci] = _A[k, fvals[ok]]
        WB[ok, ci] = _B[k, fvals[ok]]
    colidx = {kf: ci for ci, kf in enumerate(cols)}
    return WA, WB, colidx


@lru_cache(maxsize=1)
def _build():
    import concourse.mybir as mybir
    from concourse import bacc
    from concourse.tile import TileContext

    WA_np, WB_np, colidx = _packed_weights()
    ncols = WA_np.shape[1]
    f32 = mybir.dt.float32
    Copy = mybir.ActivationFunctionType.Copy

    # shift matrix: out[p] = x[p+1] (superdiagonal). Row 127 (the next block's
    # row 0) stays zero on device; the host patches that single b-term row per
    # (k, fo) — a second full-width matmul per block just to fill one row would
    # double TensorE time.
    SA_np = np.zeros((P, P), np.float32)
    for p in range(P - 1):
        SA_np[p + 1, p] = 1.0

    nc = bacc.Bacc()
    # input pretransposed by host: [s, p, j, t] = x[s, 128*(j+1) + p, t]
    x_in = nc.dram_tensor("input", [NS, P, NBL, T], f32, kind="ExternalInput")
    # output blocks for k = 1..4: [k-1, s, p, fo, t] = out_k[128*fo - r_k + p, t]
    y_out = nc.dram_tensor("output", [K - 1, NS, P, NB, T], f32,
                           kind="ExternalOutput")
    wa_dram = nc.inline_tensor(WA_np, name="wa_const")
    wb_dram = nc.inline_tensor(WB_np, name="wb_const")
    sa_dram = nc.inline_tensor(SA_np, name="sa_const")

    with TileContext(nc) as tc:
        with tc.tile_pool(name="main", bufs=1) as pool, \
             tc.tile_pool(name="ps", bufs=1, space="PSUM") as psum:
            WA = pool.tile([P, ncols], f32, tag="wa", bufs=1, name="WA")
            WB = pool.tile([P, ncols], f32, tag="wb", bufs=1, name="WB")
            SA = pool.tile([P, P], f32, tag="sa", bufs=1, name="SA")
            SCR = pool.tile([32, 1], f32, tag="scr", bufs=2, name="SCR")
            SCRA = pool.tile([32, 1], f32, tag="scra", bufs=2, name="SCRA")
            nc.sync.dma_start(out=WA[:, :], in_=wa_dram[:, :])
            nc.sync.dma_start(out=WB[:, :], in_=wb_dram[:, :])
            nc.sync.dma_start(out=SA[:, :], in_=sa_dram[:, :])
            # advance DVE/ACT/PE clocks past the const-load DMA lanes so steady
            # state compute never re-waits them (ISA sync-wait slots are scarce)
            nc.vector.tensor_copy(SCR[:, :], WA[0:32, 0:1])
            nc.scalar.copy(SCRA[:, :], WB[0:32, 0:1])

            for s in range(NS):
                X = pool.tile([P, NBL * T], f32, tag="x", bufs=3, name=f"X{s}")
                # one contiguous 14 KB descriptor per partition
                nc.sync.dma_start(
                    out=X[:, :].rearrange("p (j t) -> p j t", t=T),
                    in_=x_in[s],
                )
                # per-slice pretouch: pull the X-load DMA lane into DVE's clock
                SCX = pool.tile([32, 1], f32, tag="scr", bufs=2, name=f"SCX{s}")
                nc.vector.tensor_copy(SCX[:, :], X[0:32, 0:1])

                OT = {
                    k: pool.tile([P, NBLK[k] * T], f32, tag="o", bufs=6,
                                 name=f"O{s}_{k}")
                    for k in range(1, K)
                }
                # j-major: each shifted PSUM column is consumed by all 4
                # shifts right after its matmul, so banks recycle quickly
                for j in range(NBL):
                    PSJ = psum.tile([P, T], f32, tag="x1p", bufs=4,
                                    name=f"PS{s}_{j}")
                    nc.tensor.matmul(PSJ[:, :], SA[:, :],
                                     X[:, j * T:(j + 1) * T],
                                     start=True, stop=True)
                    for k in range(1, K):
                        q, nblk = Q[k], NBLK[k]
                        fo = j - q + 1
                        if not (0 <= fo < nblk):
                            continue
                        ci = colidx[(k, fo)]
                        dst = slice(fo * T, (fo + 1) * T)
                        BT = pool.tile([P, T], f32, tag="bt", bufs=6,
                                       name=f"BT{s}_{j}_{k}")
                        nc.scalar.activation(
                            BT[:, :], PSJ[:, :], Copy,
                            scale=WB[:, ci:ci + 1],
                        )
                        nc.vector.scalar_tensor_tensor(
                            OT[k][:, dst], X[:, j * T:(j + 1) * T],
                            WA[:, ci:ci + 1], BT[:, :],
                            mybir.AluOpType.mult, mybir.AluOpType.add,
                        )
                for k in range(1, K):
                    nc.sync.dma_start(
                        out=y_out[k - 1, s, :, 0:NBLK[k]],
                        in_=OT[k][:, :].rearrange("p (j t) -> p j t", t=T),
                    )
    nc.compile()
    return nc


def _make_in_maps(x64: np.ndarray):
    # X blocks 1..7, pretransposed to [s, p, j, t]
    xt = np.ascontiguousarray(
        x64[:, P:].reshape(64, NBL, P, T).transpose(0, 2, 1, 3))
    return [{"input": xt[NS * i:NS * (i + 1)]} for i in range(NCORES)]


def _run(x64: np.ndarray, trace: bool = False):
    from concourse.bass_utils import run_bass_kernel_spmd

    nc = _build()
    return run_bass_kernel_spmd(nc, _make_in_maps(x64), list(range(NCORES)),
                                trace=trace)


def _assemble(x: np.ndarray, results) -> np.ndarray:
    out = np.zeros((B_DIM, K, 2, NS, FR, T), np.float32)
    out[:, 0] = x.reshape(B_DIM, 2, NS, FR, T)          # k = 0: exact copy
    xr = x.reshape(B_DIM, 2, NS, FR, T)
    R_ = np.stack([results[i]["output"] for i in range(NCORES)])
    # R_[i, kk, j, p, fo, t]; core i = 2b + h, channel = 16(kk+1) + 8h + j
    for k in range(1, K):
        m, r, q, nblk = int(_M[k]), R[k], Q[k], NBLK[k]
        V = R_[:, k - 1, :, :, 0:nblk]                   # [8, NS, P, nblk, T]
        G = V.transpose(0, 1, 3, 2, 4).reshape(NCORES, NS, nblk * P, T)
        out[:, k, :, :, 0:FR - m] = (
            G[:, :, r:r + FR - m].reshape(B_DIM, 2, NS, FR - m, T))
        # device leaves the cross-block row (partition 127) of the shifted
        # operand zero; add its b-term here: out[f] += B[k,f] * x[f + m + 1]
        # at f = 128*fo - r + 127 (same f32 add order as the reference)
        for fo in range(nblk - 1):
            f = P * fo - r + P - 1
            out[:, k, :, :, f] += _B[k, f] * xr[:, :, :, f + m + 1]
    return np.ascontiguousarray(
        out.reshape(B_DIM, K * C_DIM, FR, T))


def kernel(input: np.ndarray) -> np.ndarray:
    x = np.asarray(input)
    assert x.shape == (B_DIM, C_DIM, FR, T) and x.dtype == np.float32
    x64 = x.reshape(B_DIM * C_DIM, FR, T)
    res = _run(x64)
    return _assemble(x, res.results)


# revision 16
# speedup vs baseline: 1.5666x; 1.5666x over previous
"""LogHarmonicLowering Trainium2 kernel.

Computes out[b, k*C + c, f, t] = W1[k,f] * x[b, c, i0[k,f], t] + W2[k,f] * x[b, c, i0[k,f]+1, t]
for K=5 log-harmonic frequency shifts, where i0/W1/W2 replicate the reference's
float32 fractional-shift linear interpolation (with zero padding past the top
of the frequency axis).

Because each shift k is a constant scalar, i0[k,f] = f + m_k (+1 at rare f32
rounding anomalies, folded into the weight tables), so each output block is an
integer row shift plus a constant-per-f blend of two adjacent frequency rows.
k = 0 is an exact copy, done host-side; the device computes k = 1..4.

Distribution: pure data parallel. The 64 (b, c) slices are split 8 per core.
Per core/slice, frequency lives on SBUF partitions as blocks of 128 rows with
T=512 on the free dim. Output tiles for shift k are realigned by r_k = m_k % 128
so the main operand is tile-aligned; the +1-row-shifted operand X1 is built on
the otherwise-idle TensorEngine as a bit-exact fp32 permutation matmul into
PSUM (compute engines cannot do partition-misaligned SBUF access, and re-reading
the shifted copy from HBM would add 14 MiB/core to a DMA-bound kernel).
All HBM arrays are laid out [.., partition, block, t] so every DMA is one
contiguous ~14 KB descriptor per partition (descriptor-generation bound
otherwise); the host pre/post transposes, which is cheap next to device time.
Per (slice, k, block): ScalarE computes bt = B ⊙ X1 (per-partition weights),
VectorE computes out = (X ⊙ A) + bt in one fused scalar_tensor_tensor op.
"""

import numpy as np
from functools import lru_cache

P = 128          # SBUF partitions
FR = 1024        # frequency bins
T = 512          # time steps
NB = FR // P     # 8 frequency blocks per slice
NBL = NB - 1     # blocks 1..7 are the only ones device kernels read
NS = 8           # (b, c) slices per core
NCORES = 8
B_DIM, C_DIM, K = 4, 16, 5

F_KERNEL_SIZE = 5
ANCHOR = 1
OUT_LOG_SCALE = 200.0
IN_LOG_SCALE = 0.001


def _shifts_f32() -> np.ndarray:
    """Reference's make_log_shift(), cast to float32 exactly as jnp.asarray does."""
    np_shift = (np.arange(F_KERNEL_SIZE) + 1) / ANCHOR
    log_shift = OUT_LOG_SCALE * np.log(IN_LOG_SCALE * np_shift)
    log_shift -= log_shift[ANCHOR - 1]
    return (-log_shift).astype(np.float32)


def _weight_tables():
    """Per-f effective weights A, B (float32) such that
    out[f] = A[k,f] * x[f + m_k] + B[k,f] * x[f + m_k + 1],
    bit-replicating the reference's f32 pos/floor/mask arithmetic."""
    shifts = _shifts_f32()                                     # (K,)
    f = np.arange(FR, dtype=np.float32)
    pos = f[None, :] - shifts[:, None]                         # f32 (K, FR)
    i0 = np.floor(pos).astype(np.int32)
    w = (pos - i0.astype(np.float32)).astype(np.float32)
    v0 = ((i0 >= 0) & (i0 < FR)).astype(np.float32)
    v1 = ((i0 + 1 >= 0) & (i0 + 1 < FR)).astype(np.float32)
    w1 = ((np.float32(1.0) - w) * v0).astype(np.float32)       # weight of x[i0]
    w2 = (w * v1).astype(np.float32)                           # weight of x[i0+1]

    m = i0[:, 0].copy()                                        # integer base shift per k
    delta = i0 - (np.arange(FR, dtype=np.int64)[None, :] + m[:, None])
    assert np.all((delta == 0) | (delta == 1))
    # where the f32 sum rounded up to the next integer, w == 0 exactly:
    assert np.all(w2[delta == 1] == 0.0)
    A = np.where(delta == 0, w1, np.float32(0.0)).astype(np.float32)
    B = np.where(delta == 0, w2, w1).astype(np.float32)
    assert m[0] == 0 and np.all(A[0] == 1.0) and np.all(B[0] == 0.0)
    return m, A, B


_M, _A, _B = _weight_tables()
Q = [int(_M[k]) // P for k in range(K)]       # [0, 1, 1, 2, 2]
R = [int(_M[k]) % P for k in range(K)]        # [0, 10, 91, 21, 65]
# number of output blocks computed per k (k >= 1): source block fo+q must be < NB
NBLK = [NB - Q[k] for k in range(K)]


def _packed_weights():
    """Pack A/B into [P, ncols] tiles: column (k, fo) holds the weight for
    output partition p at f = 128*fo - r_k + p (zero outside [0, FR))."""
    cols = [(k, fo) for k in range(1, K) for fo in range(NBLK[k])]
    WA = np.zeros((P, len(cols)), dtype=np.float32)
    WB = np.zeros((P, len(cols)), dtype=np.float32)
    for ci, (k, fo) in enumerate(cols):
        fvals = P * fo - R[k] + np.arange(P)
        ok = (fvals >= 0) & (fvals < FR)
        WA[ok, ci] = _A[k, fvals[ok]]
        WB[ok, ci] = _B[k, fvals[ok]]
    colidx = {kf: ci for ci, kf in enumerate(cols)}
    return WA, WB, colidx


@lru_cache(maxsize=1)
def _build():
    import concourse.mybir as mybir
    from concourse import bacc
    from concourse.tile import TileContext

    WA_np, WB_np, colidx = _packed_weights()
    ncols = WA_np.shape[1]
    f32 = mybir.dt.float32
    Copy = mybir.ActivationFunctionType.Copy

    # shift matrix: out[p] = x[p+1] (superdiagonal). Row 127 (the next block's
    # row 0) stays zero on device; the host patches that single b-term row per
    # (k, fo) — a second full-width matmul per block just to fill one row would
    # double TensorE time.
    SA_np = np.zeros((P, P), np.float32)
    for p in range(P - 1):
        SA_np[p + 1, p] = 1.0

    nc = bacc.Bacc()
    # input pretransposed by host: [s, p, j, t] = x[s, 128*(j+1) + p, t]
    x_in = nc.dram_tensor("input", [NS, P, NBL, T], f32, kind="ExternalInput")
    # output blocks for k = 1..4: [k-1, s, p, fo, t] = out_k[128*fo - r_k + p, t]
    y_out = nc.dram_tensor("output", [K - 1, NS, P, NB, T], f32,
                           kind="ExternalOutput")
    wa_dram = nc.inline_tensor(WA_np, name="wa_const")
    wb_dram = nc.inline_tensor(WB_np, name="wb_const")
    sa_dram = nc.inline_tensor(SA_np, name="sa_const")

    with TileContext(nc) as tc:
        with tc.tile_pool(name="main", bufs=1) as pool, \
             tc.tile_pool(name="ps", bufs=1, space="PSUM") as psum:
            WA = pool.tile([P, ncols], f32, tag="wa", bufs=1, name="WA")
            WB = pool.tile([P, ncols], f32, tag="wb", bufs=1, name="WB")
            SA = pool.tile([P, P], f32, tag="sa", bufs=1, name="SA")
            SCR = pool.tile([32, 1], f32, tag="scr", bufs=2, name="SCR")
            SCRA = pool.tile([32, 1], f32, tag="scra", bufs=2, name="SCRA")
            nc.sync.dma_start(out=WA[:, :], in_=wa_dram[:, :])
            nc.sync.dma_start(out=WB[:, :], in_=wb_dram[:, :])
            nc.sync.dma_start(out=SA[:, :], in_=sa_dram[:, :])
            # advance DVE/ACT/PE clocks past the const-load DMA lanes so steady
            # state compute never re-waits them (ISA sync-wait slots are scarce)
            nc.vector.tensor_copy(SCR[:, :], WA[0:32, 0:1])
            nc.scalar.copy(SCRA[:, :], WB[0:32, 0:1])

            for s in range(NS):
                X = pool.tile([P, NBL * T], f32, tag="x", bufs=4, name=f"X{s}")
                # one contiguous 14 KB descriptor per partition
                nc.sync.dma_start(
                    out=X[:, :].rearrange("p (j t) -> p j t", t=T),
                    in_=x_in[s],
                )
                # per-slice pretouch: pull the X-load DMA lane into DVE's clock
                SCX = pool.tile([32, 1], f32, tag="scr", bufs=2, name=f"SCX{s}")
                nc.vector.tensor_copy(SCX[:, :], X[0:32, 0:1])

                # X1[j] = row-shifted x block j+1, built on the TensorEngine
                X1 = []
                for j in range(NBL):
                    PSJ = psum.tile([P, T], f32, tag="x1p", bufs=8,
                                    name=f"PS{s}_{j}")
                    nc.tensor.matmul(PSJ[:, :], SA[:, :],
                                     X[:, j * T:(j + 1) * T],
                                     start=True, stop=True)
                    X1.append(PSJ)

                for k in range(1, K):
                    q, nblk = Q[k], NBLK[k]
                    BT = pool.tile([P, nblk * T], f32, tag="bt", bufs=3,
                                   name=f"BT{s}_{k}")
                    O = pool.tile([P, nblk * T], f32, tag="o", bufs=4,
                                  name=f"O{s}_{k}")
                    for fo in range(nblk):
                        ci = colidx[(k, fo)]
                        dst = slice(fo * T, (fo + 1) * T)
                        src = slice((fo + q - 1) * T, (fo + q) * T)
                        nc.scalar.activation(
                            BT[:, dst], X1[fo + q - 1][:, :], Copy,
                            scale=WB[:, ci:ci + 1],
                        )
                        nc.vector.scalar_tensor_tensor(
                            O[:, dst], X[:, src], WA[:, ci:ci + 1], BT[:, dst],
                            mybir.AluOpType.mult, mybir.AluOpType.add,
                        )
                    nc.sync.dma_start(
                        out=y_out[k - 1, s, :, 0:nblk],
                        in_=O[:, :].rearrange("p (j t) -> p j t", t=T),
                    )
    nc.compile()
    return nc


def _make_in_maps(x64: np.ndarray):
    # X blocks 1..7, pretransposed to [s, p, j, t]
    xt = np.ascontiguousarray(
        x64[:, P:].reshape(64, NBL, P, T).transpose(0, 2, 1, 3))
    return [{"input": xt[NS * i:NS * (i + 1)]} for i in range(NCORES)]


def _run(x64: np.ndarray, trace: bool = False):
    from concourse.bass_utils import run_bass_kernel_spmd

    nc = _build()
    return run_bass_kernel_spmd(nc, _make_in_maps(x64), list(range(NCORES)),
                                trace=trace)


def _assemble(x: np.ndarray, results) -> np.ndarray:
    out = np.zeros((B_DIM, K, 2, NS, FR, T), np.float32)
    out[:, 0] = x.reshape(B_DIM, 2, NS, FR, T)          # k = 0: exact copy
    xr = x.reshape(B_DIM, 2, NS, FR, T)
    R_ = np.stack([results[i]["output"] for i in range(NCORES)])
    # R_[i, kk, j, p, fo, t]; core i = 2b + h, channel = 16(kk+1) + 8h + j
    for k in range(1, K):
        m, r, q, nblk = int(_M[k]), R[k], Q[k], NBLK[k]
        V = R_[:, k - 1, :, :, 0:nblk]                   # [8, NS, P, nblk, T]
        G = V.transpose(0, 1, 3, 2, 4).reshape(NCORES, NS, nblk * P, T)
        out[:, k, :, :, 0:FR - m] = (
            G[:, :, r:r + FR - m].reshape(B_DIM, 2, NS, FR - m, T))
        # device leaves the cross-block row (partition 127) of the shifted
        # operand zero; add its b-term here: out[f] += B[k,f] * x[f + m + 1]
        # at f = 128*fo - r + 127 (same f32 add order as the reference)
        for fo in range(nblk - 1):
            f = P * fo - r + P - 1
            out[:, k, :, :, f] += _B[k, f] * xr[:, :, :, f + m + 1]
    return np.ascontiguousarray(
        out.reshape(B_DIM, K * C_DIM, FR, T))


def kernel(input: np.ndarray) -> np.ndarray:
    x = np.asarray(input)
    assert x.shape == (B_DIM, C_DIM, FR, T) and x.dtype == np.float32
    x64 = x.reshape(B_DIM * C_DIM, FR, T)
    res = _run(x64)
    return _assemble(x, res.results)
